# revision 9
# baseline (speedup 1.0000x reference)
"""Trainium2 Bass kernel for nn_CustomBSplineLayer.

Math: out[b,o] = sum_{i,g} coeff[o,i,g] * w[o,i] * s_g(clip(x[b,i], -1, 1))
where s_g is a cubic B-spline basis (integer knots in t = 3.5*(x+1) space).

Truncated-power identity: V_q = relu(t-q)^3 (q=0..6) spans all s_g on [0,7],
so out = sum_{q,i} P_q[b,i] * H[(q,i), o] for any plane basis P spanning
{V_q} (H solved exactly on host).  Plane basis (levels 2,2,1,0,0,0,0):
    P = {W2_0, W2_1, D1_2, V_3, V_4, V_5, V_6}     (max magnitudes
    36, 30, 61, 64, 27, 8, 1)
with D1_q = V_q - V_{q+1}, W2_q = D1_q - D1_{q+1}.

Precision design (gate is 2e-2; this lands ~4.7e-3 in exact simulation):
  - planes and H are fp16: fp16 x fp16 matmul products are EXACT in f32
    PSUM (PE full rate, same as f32r, minus f32r's 2^-12 product rounding).
  - the fold chain q<=3 (big cube values, catastrophic cancellation if
    quantized early) stays fp32 end-to-end; only the final small-magnitude
    plane values round to fp16.
  - the q>=4 cubes run all-fp16 (V = a*a*a with a = relu(t-q) small), which
    makes those DVE ops 2-4x faster (dve 2x/4x perf modes need 2-byte
    dtypes).
  - tpc = clip(3.5 x, -3.5, 3.5) fp16 on host; its quantization enters all
    planes through the same delta-t, and every plane has small d/dt.

Engine split (per i-block): ACT does 7 relus + 2 squares + 1 cast copy,
DVE does 2 f32 squares + 4 f32 cubes + 2 f32->f16 fold subs + 1 cast +
6 fp16 square/cube ops, GpSimd does the 3 f32 D1 subs.  PSUM drains
alternate ACT/DVE.  Production order q = 5,6,4,3,2,1,0 puts cheap fp16
planes first so the PE starts ~2.5us in and then never starves — the PE
only reaches its full 2.4 GHz pstate after ~3us of CONTINUOUS execution,
so any gap halves matmul throughput.

Layout (data-parallel over batch, 8 cores x 1024 rows):
  - xt [512 i, 1024 b] fp16 per core (i on partitions), relu bias 3.5-q.
  - planes per (i-block, q): [128, 1024] fp16; matmul lhsT slices are
    [128 K, 128 M=batch]; rhs H tiles [128, 512 o] fp16.
  - PSUM [128 b, 512 o] f32 x 8 banks accumulate all 28 k-tiles; each bank
    drains right after its final matmul.
"""

import numpy as np

import concourse.mybir as mybir
from concourse import bacc
import concourse.tile as tile
from concourse.bass_utils import run_bass_kernel_spmd

F32 = mybir.dt.float32
F16 = mybir.dt.float16
AOT = mybir.AluOpType
ACTF = mybir.ActivationFunctionType

N_CORES = 8
BATCH, I, O, G = 8192, 512, 512, 8
BC = BATCH // N_CORES          # 1024 batch rows per core
Q = 7                          # planes q = 0..6
IB = I // 128                  # 4 i-blocks
KT = Q * IB                    # 28 k-tiles of 128
NBB = BC // 128                # 8 batch blocks of 128

LEVELS = (2, 2, 1, 0, 0, 0, 0)

_programs = {}


def _build_program(knobs=0):
    nc = bacc.Bacc("TRN2", target_bir_lowering=False, debug=False,
                   num_devices=N_CORES)
    xt_d = nc.dram_tensor("xt", [I, BC], F16, kind="ExternalInput").ap()
    h2_d = nc.dram_tensor("h2", [KT * 128, O], F16, kind="ExternalInput").ap()
    qb_d = nc.dram_tensor("qb", [128, 8], F32, kind="ExternalInput").ap()
    out_d = nc.dram_tensor("out", [BC, O], F32, kind="ExternalOutput").ap()

    with tile.TileContext(nc) as tc:
        with tc.tile_pool(name="g", bufs=1) as gpool, \
             tc.tile_pool(name="x", bufs=2) as xpool, \
             tc.tile_pool(name="af", bufs=5) as afpool, \
             tc.tile_pool(name="ah", bufs=3) as ahpool, \
             tc.tile_pool(name="sf", bufs=5) as sfpool, \
             tc.tile_pool(name="sh", bufs=3) as shpool, \
             tc.tile_pool(name="vf", bufs=6) as vfpool, \
             tc.tile_pool(name="df", bufs=4) as dfpool, \
             tc.tile_pool(name="p", bufs=12) as ppool, \
             tc.tile_pool(name="o", bufs=4) as opool, \
             tc.tile_pool(name="ps", bufs=1, space="PSUM") as pspool:

            qb_s = gpool.tile([128, 8], F32)
            nc.sync.dma_start(out=qb_s[:], in_=qb_d[:])

            h2_s = gpool.tile([128, KT, O], F16)
            for ib in range(IB):
                nc.sync.dma_start(
                    out=h2_s[:, ib * Q:(ib + 1) * Q, :],
                    in_=h2_d[ib * Q * 128:(ib + 1) * Q * 128, :].rearrange(
                        "(kt p) o -> p kt o", p=128))

            psums = [pspool.tile([128, O], F32, name=f"ps{bb}", tag=f"ps{bb}")
                     for bb in range(NBB)]

            issue = {"n": 0}

            def mk_plane(ib, q, lhs):
                kt = ib * Q + q
                rhs = h2_s[:, kt, :]
                first = issue["n"] == 0
                last = issue["n"] == KT - 1
                issue["n"] += 1
                for bb in range(NBB):
                    nc.tensor.matmul(psums[bb][:],
                                     lhs[:, bb * 128:(bb + 1) * 128],
                                     rhs,
                                     start=first, stop=last)
                    if last:
                        o = opool.tile([128, O], F32, tag="o")
                        if bb % 4 != 3:
                            nc.scalar.copy(o[:], psums[bb][:])
                        else:
                            nc.vector.tensor_copy(out=o[:], in_=psums[bb][:])
                        nc.sync.dma_start(
                            out=out_d[bb * 128:(bb + 1) * 128, :], in_=o[:])

            def head(ib):
                """Issue all ops of i-block ib that do not depend on the
                slow GpSimd D1 chain: fp16 planes q=5,6,4, the fp16-rounded
                V3 plane, the f32 cubes, and the GP D1 subs themselves."""
                xs = xpool.tile([128, BC], F16, tag="x")
                nc.sync.dma_start(out=xs[:], in_=xt_d[ib * 128:(ib + 1) * 128, :])

                def relu(q, dtype):
                    a = (afpool if dtype == F32 else ahpool).tile(
                        [128, BC], dtype, tag="a" + ("f" if dtype == F32 else "h"))
                    nc.scalar.activation(a[:], xs[:], ACTF.Relu,
                                         bias=qb_s[:, q:q + 1], scale=1.0)
                    return a

                def sq_act(a, dtype):
                    sq = (sfpool if dtype == F32 else shpool).tile(
                        [128, BC], dtype, tag="s" + ("f" if dtype == F32 else "h"))
                    nc.scalar.activation(sq[:], a[:], ACTF.Square)
                    return sq

                def cube16(q, sq_on_act=False):
                    a = relu(q, F16)
                    if sq_on_act:
                        sq = sq_act(a, F16)
                    else:
                        sq = shpool.tile([128, BC], F16, tag="sh")
                        nc.vector.tensor_tensor(out=sq[:], in0=a[:], in1=a[:],
                                                op=AOT.mult)
                    vq = ppool.tile([128, BC], F16, tag="pl", bufs=12,
                                    name=f"v16_{ib}_{q}")
                    nc.vector.tensor_tensor(out=vq[:], in0=sq[:], in1=a[:],
                                            op=AOT.mult)
                    return vq

                def cube32(q, out_dtype=F32, sq_on_act=False):
                    """f32 relu+square; cube on DVE (f32 or fp16-rounded out)."""
                    a = relu(q, F32)
                    if sq_on_act:
                        sq = sq_act(a, F32)
                    else:
                        sq = sfpool.tile([128, BC], F32, tag="sf")
                        nc.vector.tensor_tensor(out=sq[:], in0=a[:], in1=a[:],
                                                op=AOT.mult)
                    if out_dtype == F32:
                        vq = vfpool.tile([128, BC], F32, tag="vf", bufs=6,
                                         name=f"v32_{ib}_{q}")
                    else:
                        vq = ppool.tile([128, BC], F16, tag="pl", bufs=12,
                                        name=f"v3h_{ib}")
                    nc.vector.tensor_tensor(out=vq[:], in0=sq[:], in1=a[:],
                                            op=AOT.mult)
                    return vq

                def gp_sub(x0, x1, name):
                    d = dfpool.tile([128, BC], F32, tag="df", bufs=7,
                                    name=f"{name}_{ib}")
                    nc.gpsimd.tensor_tensor(out=d[:], in0=x0[:], in1=x1[:],
                                            op=AOT.subtract)
                    return d

                # cheap fp16 planes first so the PE starts early (squares
                # for q=5,6 on ACT, issued right after their relus so the
                # DVE cube-mults barely wait)
                mk_plane(ib, 5, cube16(5, sq_on_act=True))
                mk_plane(ib, 6, cube16(6, sq_on_act=True))
                mk_plane(ib, 4, cube16(4))
                # V3: f32 relu (ACT), f32 square (ACT, ready before DVE
                # needs it), fp16-rounded cube IS plane 3 and also feeds
                # d1_2 (quantization 64*2^-12 on |D1_2|<=61 is 3e-4)
                v3h = cube32(3, out_dtype=F16, sq_on_act=True)
                mk_plane(ib, 3, v3h)
                v2 = cube32(2)
                # GP writes the D1_2 plane in fp16 directly (|D1_2| <= 61)
                p2 = ppool.tile([128, BC], F16, tag="pl", bufs=12,
                                name=f"p2_{ib}")
                nc.gpsimd.tensor_tensor(out=p2[:], in0=v2[:], in1=v3h[:],
                                        op=AOT.subtract)
                v1 = cube32(1, sq_on_act=True)
                d1_1 = gp_sub(v1, v2, "d1_1")
                v0 = cube32(0, sq_on_act=True)
                d1_0 = gp_sub(v0, v1, "d1_0")
                return ib, p2, d1_0, d1_1

            def tail(st):
                """Fold planes of an earlier i-block, issued after the next
                head so neither DVE nor the PE stalls on the GP chain."""
                ib, p2, d1_0, d1_1 = st
                mk_plane(ib, 2, p2)
                p1 = ppool.tile([128, BC], F16, tag="pl", bufs=12,
                                name=f"p1_{ib}")
                nc.vector.tensor_tensor(out=p1[:], in0=d1_1[:], in1=p2[:],
                                        op=AOT.subtract)
                mk_plane(ib, 1, p1)
                p0 = ppool.tile([128, BC], F16, tag="pl", bufs=12,
                                name=f"p0_{ib}")
                nc.vector.tensor_tensor(out=p0[:], in0=d1_0[:], in1=d1_1[:],
                                        op=AOT.subtract)
                mk_plane(ib, 0, p0)

            prev = None
            for ib in range(IB):
                st = head(ib)
                if prev is not None:
                    tail(prev)
                prev = st
            tail(prev)

    nc.compile()
    return nc


def _get_program(knobs=0):
    if knobs not in _programs:
        _programs[knobs] = _build_program(knobs)
    return _programs[knobs]


_STENS = {0: (1.0,), 1: (1.0, -1.0), 2: (1.0, -2.0, 1.0)}


def _host_prep(x, weights, coefficients, levels=LEVELS):
    x = np.ascontiguousarray(np.asarray(x, dtype=np.float32))
    weights = np.asarray(weights, dtype=np.float32)
    coefficients = np.asarray(coefficients, dtype=np.float32)

    # raw truncated-power coefficients G_q = sum_g w5[q-g]/6 * C2_g  (g<=6)
    c2 = coefficients.astype(np.float64) * weights.astype(np.float64)[:, :, None]
    c2 = c2.transpose(2, 1, 0)[:Q]                 # [7, I, O]
    w5 = np.array([1.0, -4.0, 6.0, -4.0, 1.0])
    graw = np.zeros((Q, I, O), dtype=np.float64)
    for q in range(Q):
        for g in range(Q):
            r = q - g
            if 0 <= r <= 4:
                graw[q] += (w5[r] / 6.0) * c2[g]
    # planes P = A V  =>  coefficients H = A^{-T} G (exact basis change)
    A = np.zeros((Q, Q))
    for q in range(Q):
        for u, s in enumerate(_STENS[levels[q]]):
            if q + u < Q:
                A[q, q + u] = s
    h = np.einsum('pq,qio->pio', np.linalg.inv(A).T, graw)
    # device row order kt = ib*7 + q
    h2k = np.empty((KT, 128, O), dtype=np.float16)
    for ib in range(IB):
        for q in range(Q):
            h2k[ib * Q + q] = h[q, ib * 128:(ib + 1) * 128, :]
    h2k = np.ascontiguousarray(h2k.reshape(KT * 128, O))

    # tpc = clip(3.5*x, -3.5, 3.5) in t-minus-3.5 coords; relu bias is 3.5-q
    tpc = np.clip(3.5 * x, -3.5, 3.5).astype(np.float16)
    xt = np.ascontiguousarray(tpc.T)               # [I, B] fp16
    qb = np.tile((3.5 - np.arange(8, dtype=np.float32))[None, :], (128, 1))

    in_maps = []
    for c in range(N_CORES):
        in_maps.append({
            "xt": np.ascontiguousarray(xt[:, c * BC:(c + 1) * BC]),
            "h2": h2k,
            "qb": qb,
        })
    return in_maps


def _run(x, weights, coefficients, knobs=0, **spmd_kwargs):
    nc = _get_program(knobs)
    in_maps = _host_prep(x, weights, coefficients)
    res = run_bass_kernel_spmd(nc, in_maps, list(range(N_CORES)), **spmd_kwargs)
    out = np.concatenate([res.results[c]["out"] for c in range(N_CORES)], axis=0)
    return out.astype(np.float32), res


def kernel(x, weights, coefficients):
    out, _ = _run(x, weights, coefficients)
    return out


# revision 10
# speedup vs baseline: 1.0008x; 1.0008x over previous
"""Trainium2 Bass kernel for nn_CustomBSplineLayer.

Math: out[b,o] = sum_{i,g} coeff[o,i,g] * w[o,i] * s_g(clip(x[b,i], -1, 1))
where s_g is a cubic B-spline basis (integer knots in t = 3.5*(x+1) space).

Truncated-power identity: V_q = relu(t-q)^3 (q=0..6) spans all s_g on [0,7],
so out = sum_{q,i} P_q[b,i] * H[(q,i), o] for any plane basis P spanning
{V_q} (H solved exactly on host).  Plane basis (levels 2,2,1,0,0,0,0):
    P = {W2_0, W2_1, D1_2, V_3, V_4, V_5, V_6}     (max magnitudes
    36, 30, 61, 64, 27, 8, 1)
with D1_q = V_q - V_{q+1}, W2_q = D1_q - D1_{q+1}.

Precision design (gate is 2e-2; this lands ~4.7e-3 in exact simulation):
  - planes and H are fp16: fp16 x fp16 matmul products are EXACT in f32
    PSUM (PE full rate, same as f32r, minus f32r's 2^-12 product rounding).
  - the fold chain q<=3 (big cube values, catastrophic cancellation if
    quantized early) stays fp32 end-to-end; only the final small-magnitude
    plane values round to fp16.
  - the q>=4 cubes run all-fp16 (V = a*a*a with a = relu(t-q) small), which
    makes those DVE ops 2-4x faster (dve 2x/4x perf modes need 2-byte
    dtypes).
  - tpc = clip(3.5 x, -3.5, 3.5) fp16 on host; its quantization enters all
    planes through the same delta-t, and every plane has small d/dt.

Engine split (per i-block): ACT does 7 relus + 2 squares + 1 cast copy,
DVE does 2 f32 squares + 4 f32 cubes + 2 f32->f16 fold subs + 1 cast +
6 fp16 square/cube ops, GpSimd does the 3 f32 D1 subs.  PSUM drains
alternate ACT/DVE.  Production order q = 5,6,4,3,2,1,0 puts cheap fp16
planes first so the PE starts ~2.5us in and then never starves — the PE
only reaches its full 2.4 GHz pstate after ~3us of CONTINUOUS execution,
so any gap halves matmul throughput.

Layout (data-parallel over batch, 8 cores x 1024 rows):
  - xt [512 i, 1024 b] fp16 per core (i on partitions), relu bias 3.5-q.
  - planes per (i-block, q): [128, 1024] fp16; matmul lhsT slices are
    [128 K, 128 M=batch]; rhs H tiles [128, 512 o] fp16.
  - PSUM [128 b, 512 o] f32 x 8 banks accumulate all 28 k-tiles; each bank
    drains right after its final matmul.
"""

import numpy as np

import concourse.mybir as mybir
from concourse import bacc
import concourse.tile as tile
from concourse.bass_utils import run_bass_kernel_spmd

F32 = mybir.dt.float32
F16 = mybir.dt.float16
AOT = mybir.AluOpType
ACTF = mybir.ActivationFunctionType

N_CORES = 8
BATCH, I, O, G = 8192, 512, 512, 8
BC = BATCH // N_CORES          # 1024 batch rows per core
Q = 7                          # planes q = 0..6
IB = I // 128                  # 4 i-blocks
KT = Q * IB                    # 28 k-tiles of 128
NBB = BC // 128                # 8 batch blocks of 128

LEVELS = (2, 2, 1, 0, 0, 0, 0)

_programs = {}


def _build_program(knobs=0):
    nc = bacc.Bacc("TRN2", target_bir_lowering=False, debug=False,
                   num_devices=N_CORES)
    xt_d = nc.dram_tensor("xt", [I, BC], F16, kind="ExternalInput").ap()
    h2_d = nc.dram_tensor("h2", [KT * 128, O], F16, kind="ExternalInput").ap()
    qb_d = nc.dram_tensor("qb", [128, 8], F32, kind="ExternalInput").ap()
    out_d = nc.dram_tensor("out", [BC, O], F32, kind="ExternalOutput").ap()

    with tile.TileContext(nc) as tc:
        with tc.tile_pool(name="g", bufs=1) as gpool, \
             tc.tile_pool(name="x", bufs=2) as xpool, \
             tc.tile_pool(name="af", bufs=5) as afpool, \
             tc.tile_pool(name="ah", bufs=3) as ahpool, \
             tc.tile_pool(name="sf", bufs=5) as sfpool, \
             tc.tile_pool(name="sh", bufs=3) as shpool, \
             tc.tile_pool(name="vf", bufs=6) as vfpool, \
             tc.tile_pool(name="df", bufs=4) as dfpool, \
             tc.tile_pool(name="p", bufs=12) as ppool, \
             tc.tile_pool(name="o", bufs=4) as opool, \
             tc.tile_pool(name="ps", bufs=1, space="PSUM") as pspool:

            qb_s = gpool.tile([128, 8], F32)
            nc.sync.dma_start(out=qb_s[:], in_=qb_d[:])

            h2_s = gpool.tile([128, KT, O], F16)
            for ib in range(IB):
                nc.sync.dma_start(
                    out=h2_s[:, ib * Q:(ib + 1) * Q, :],
                    in_=h2_d[ib * Q * 128:(ib + 1) * Q * 128, :].rearrange(
                        "(kt p) o -> p kt o", p=128))

            psums = [pspool.tile([128, O], F32, name=f"ps{bb}", tag=f"ps{bb}")
                     for bb in range(NBB)]

            issue = {"n": 0}

            def mk_plane(ib, q, lhs):
                kt = ib * Q + q
                rhs = h2_s[:, kt, :]
                first = issue["n"] == 0
                last = issue["n"] == KT - 1
                issue["n"] += 1
                for bb in range(NBB):
                    nc.tensor.matmul(psums[bb][:],
                                     lhs[:, bb * 128:(bb + 1) * 128],
                                     rhs,
                                     start=first, stop=last)
                    if last:
                        o = opool.tile([128, O], F32, tag="o")
                        if bb % 2 == 0:
                            nc.scalar.copy(o[:], psums[bb][:])
                        else:
                            nc.vector.tensor_copy(out=o[:], in_=psums[bb][:])
                        nc.sync.dma_start(
                            out=out_d[bb * 128:(bb + 1) * 128, :], in_=o[:])

            def head(ib):
                """Issue all ops of i-block ib that do not depend on the
                slow GpSimd D1 chain: fp16 planes q=5,6,4, the fp16-rounded
                V3 plane, the f32 cubes, and the GP D1 subs themselves."""
                xs = xpool.tile([128, BC], F16, tag="x")
                nc.sync.dma_start(out=xs[:], in_=xt_d[ib * 128:(ib + 1) * 128, :])

                def relu(q, dtype):
                    a = (afpool if dtype == F32 else ahpool).tile(
                        [128, BC], dtype, tag="a" + ("f" if dtype == F32 else "h"))
                    nc.scalar.activation(a[:], xs[:], ACTF.Relu,
                                         bias=qb_s[:, q:q + 1], scale=1.0)
                    return a

                def sq_act(a, dtype):
                    sq = (sfpool if dtype == F32 else shpool).tile(
                        [128, BC], dtype, tag="s" + ("f" if dtype == F32 else "h"))
                    nc.scalar.activation(sq[:], a[:], ACTF.Square)
                    return sq

                def cube16(q, sq_on_act=False):
                    a = relu(q, F16)
                    if sq_on_act:
                        sq = sq_act(a, F16)
                    else:
                        sq = shpool.tile([128, BC], F16, tag="sh")
                        nc.vector.tensor_tensor(out=sq[:], in0=a[:], in1=a[:],
                                                op=AOT.mult)
                    vq = ppool.tile([128, BC], F16, tag="pl", bufs=12,
                                    name=f"v16_{ib}_{q}")
                    nc.vector.tensor_tensor(out=vq[:], in0=sq[:], in1=a[:],
                                            op=AOT.mult)
                    return vq

                def cube32(q, out_dtype=F32, sq_on_act=False):
                    """f32 relu+square; cube on DVE (f32 or fp16-rounded out)."""
                    a = relu(q, F32)
                    if sq_on_act:
                        sq = sq_act(a, F32)
                    else:
                        sq = sfpool.tile([128, BC], F32, tag="sf")
                        nc.vector.tensor_tensor(out=sq[:], in0=a[:], in1=a[:],
                                                op=AOT.mult)
                    if out_dtype == F32:
                        vq = vfpool.tile([128, BC], F32, tag="vf", bufs=6,
                                         name=f"v32_{ib}_{q}")
                    else:
                        vq = ppool.tile([128, BC], F16, tag="pl", bufs=12,
                                        name=f"v3h_{ib}")
                    nc.vector.tensor_tensor(out=vq[:], in0=sq[:], in1=a[:],
                                            op=AOT.mult)
                    return vq

                def gp_sub(x0, x1, name):
                    d = dfpool.tile([128, BC], F32, tag="df", bufs=7,
                                    name=f"{name}_{ib}")
                    nc.gpsimd.tensor_tensor(out=d[:], in0=x0[:], in1=x1[:],
                                            op=AOT.subtract)
                    return d

                # cheap fp16 planes first so the PE starts early
                mk_plane(ib, 5, cube16(5))
                mk_plane(ib, 6, cube16(6))
                mk_plane(ib, 4, cube16(4))
                # V3: f32 relu (ACT), f32 square (ACT, ready before DVE
                # needs it), fp16-rounded cube IS plane 3 and also feeds
                # d1_2 (quantization 64*2^-12 on |D1_2|<=61 is 3e-4)
                v3h = cube32(3, out_dtype=F16, sq_on_act=True)
                mk_plane(ib, 3, v3h)
                v2 = cube32(2)
                # GP writes the D1_2 plane in fp16 directly (|D1_2| <= 61)
                p2 = ppool.tile([128, BC], F16, tag="pl", bufs=12,
                                name=f"p2_{ib}")
                nc.gpsimd.tensor_tensor(out=p2[:], in0=v2[:], in1=v3h[:],
                                        op=AOT.subtract)
                v1 = cube32(1, sq_on_act=True)
                d1_1 = gp_sub(v1, v2, "d1_1")
                v0 = cube32(0, sq_on_act=True)
                d1_0 = gp_sub(v0, v1, "d1_0")
                return ib, p2, d1_0, d1_1

            def tail(st):
                """Fold planes of an earlier i-block, issued after the next
                head so neither DVE nor the PE stalls on the GP chain."""
                ib, p2, d1_0, d1_1 = st
                mk_plane(ib, 2, p2)
                p1 = ppool.tile([128, BC], F16, tag="pl", bufs=12,
                                name=f"p1_{ib}")
                nc.vector.tensor_tensor(out=p1[:], in0=d1_1[:], in1=p2[:],
                                        op=AOT.subtract)
                mk_plane(ib, 1, p1)
                p0 = ppool.tile([128, BC], F16, tag="pl", bufs=12,
                                name=f"p0_{ib}")
                nc.vector.tensor_tensor(out=p0[:], in0=d1_0[:], in1=d1_1[:],
                                        op=AOT.subtract)
                mk_plane(ib, 0, p0)

            prev = None
            for ib in range(IB):
                st = head(ib)
                if prev is not None:
                    tail(prev)
                prev = st
            tail(prev)

    nc.compile()
    return nc


def _get_program(knobs=0):
    if knobs not in _programs:
        _programs[knobs] = _build_program(knobs)
    return _programs[knobs]


_STENS = {0: (1.0,), 1: (1.0, -1.0), 2: (1.0, -2.0, 1.0)}


def _host_prep(x, weights, coefficients, levels=LEVELS):
    x = np.ascontiguousarray(np.asarray(x, dtype=np.float32))
    weights = np.asarray(weights, dtype=np.float32)
    coefficients = np.asarray(coefficients, dtype=np.float32)

    # raw truncated-power coefficients G_q = sum_g w5[q-g]/6 * C2_g  (g<=6)
    c2 = coefficients.astype(np.float64) * weights.astype(np.float64)[:, :, None]
    c2 = c2.transpose(2, 1, 0)[:Q]                 # [7, I, O]
    w5 = np.array([1.0, -4.0, 6.0, -4.0, 1.0])
    graw = np.zeros((Q, I, O), dtype=np.float64)
    for q in range(Q):
        for g in range(Q):
            r = q - g
            if 0 <= r <= 4:
                graw[q] += (w5[r] / 6.0) * c2[g]
    # planes P = A V  =>  coefficients H = A^{-T} G (exact basis change)
    A = np.zeros((Q, Q))
    for q in range(Q):
        for u, s in enumerate(_STENS[levels[q]]):
            if q + u < Q:
                A[q, q + u] = s
    h = np.einsum('pq,qio->pio', np.linalg.inv(A).T, graw)
    # device row order kt = ib*7 + q
    h2k = np.empty((KT, 128, O), dtype=np.float16)
    for ib in range(IB):
        for q in range(Q):
            h2k[ib * Q + q] = h[q, ib * 128:(ib + 1) * 128, :]
    h2k = np.ascontiguousarray(h2k.reshape(KT * 128, O))

    # tpc = clip(3.5*x, -3.5, 3.5) in t-minus-3.5 coords; relu bias is 3.5-q
    tpc = np.clip(3.5 * x, -3.5, 3.5).astype(np.float16)
    xt = np.ascontiguousarray(tpc.T)               # [I, B] fp16
    qb = np.tile((3.5 - np.arange(8, dtype=np.float32))[None, :], (128, 1))

    in_maps = []
    for c in range(N_CORES):
        in_maps.append({
            "xt": np.ascontiguousarray(xt[:, c * BC:(c + 1) * BC]),
            "h2": h2k,
            "qb": qb,
        })
    return in_maps


def _run(x, weights, coefficients, knobs=0, **spmd_kwargs):
    nc = _get_program(knobs)
    in_maps = _host_prep(x, weights, coefficients)
    res = run_bass_kernel_spmd(nc, in_maps, list(range(N_CORES)), **spmd_kwargs)
    out = np.concatenate([res.results[c]["out"] for c in range(N_CORES)], axis=0)
    return out.astype(np.float32), res


def kernel(x, weights, coefficients):
    out, _ = _run(x, weights, coefficients)
    return out


# revision 13
# speedup vs baseline: 1.1312x; 1.1303x over previous
"""Trainium2 Bass kernel for nn_CustomBSplineLayer.

Math: out[b,o] = sum_{i,g} coeff[o,i,g] * w[o,i] * s_g(clip(x[b,i], -1, 1))
where s_g is a cubic B-spline basis (integer knots in t = 3.5*(x+1) space).

Truncated-power identity: V_q = relu(t-q)^3 (q=0..6) spans all s_g on [0,7],
so out = sum_{q,i} P_q[b,i] * H[(q,i), o] for any plane basis P spanning
{V_q} (H solved exactly on host).  Plane basis (levels 2,2,1,0,0,0,0):
    P = {W2_0, W2_1, D1_2, V_3, V_4, V_5, V_6}     (max magnitudes
    36, 30, 61, 64, 27, 8, 1)
with D1_q = V_q - V_{q+1}, W2_q = D1_q - D1_{q+1}.

Precision design (gate is 2e-2; this lands ~4.7e-3 in exact simulation):
  - planes and H are fp16: fp16 x fp16 matmul products are EXACT in f32
    PSUM (PE full rate, same as f32r, minus f32r's 2^-12 product rounding).
  - the fold chain q<=3 (big cube values, catastrophic cancellation if
    quantized early) stays fp32 end-to-end; only the final small-magnitude
    plane values round to fp16.
  - the q>=4 cubes run all-fp16 (V = a*a*a with a = relu(t-q) small), which
    makes those DVE ops 2-4x faster (dve 2x/4x perf modes need 2-byte
    dtypes).
  - tpc = clip(3.5 x, -3.5, 3.5) fp16 on host; its quantization enters all
    planes through the same delta-t, and every plane has small d/dt.

Engine split (per i-block): ACT does 7 relus + 2 squares + 1 cast copy,
DVE does 2 f32 squares + 4 f32 cubes + 2 f32->f16 fold subs + 1 cast +
6 fp16 square/cube ops, GpSimd does the 3 f32 D1 subs.  PSUM drains
alternate ACT/DVE.  Production order q = 5,6,4,3,2,1,0 puts cheap fp16
planes first so the PE starts ~2.5us in and then never starves — the PE
only reaches its full 2.4 GHz pstate after ~3us of CONTINUOUS execution,
so any gap halves matmul throughput.

Layout (data-parallel over batch, 8 cores x 1024 rows):
  - xt [512 i, 1024 b] fp16 per core (i on partitions), relu bias 3.5-q.
  - planes per (i-block, q): [128, 1024] fp16; matmul lhsT slices are
    [128 K, 128 M=batch]; rhs H tiles [128, 512 o] fp16.
  - PSUM [128 b, 512 o] f32 x 8 banks accumulate all 28 k-tiles; each bank
    drains right after its final matmul.
"""

import numpy as np

import concourse.mybir as mybir
from concourse import bacc
import concourse.tile as tile
from concourse.bass_utils import run_bass_kernel_spmd

F32 = mybir.dt.float32
F16 = mybir.dt.float16
AOT = mybir.AluOpType
ACTF = mybir.ActivationFunctionType

N_CORES = 8
BATCH, I, O, G = 8192, 512, 512, 8
BC = BATCH // N_CORES          # 1024 batch rows per core
Q = 7                          # planes q = 0..6
IB = I // 128                  # 4 i-blocks
KT = Q * IB                    # 28 k-tiles of 128
NBB = BC // 128                # 8 batch blocks of 128

LEVELS = (2, 2, 1, 0, 0, 0, 0)

_programs = {}


def _build_program(knobs=0):
    nc = bacc.Bacc("TRN2", target_bir_lowering=False, debug=False,
                   num_devices=N_CORES)
    xt_d = nc.dram_tensor("xt", [I, BC], F16, kind="ExternalInput").ap()
    h2_d = nc.dram_tensor("h2", [KT * 128, O], F16, kind="ExternalInput").ap()
    qb_d = nc.dram_tensor("qb", [128, 8], F32, kind="ExternalInput").ap()
    out_d = nc.dram_tensor("out", [BC, O], F32, kind="ExternalOutput").ap()

    with tile.TileContext(nc) as tc:
        with tc.tile_pool(name="g", bufs=1) as gpool, \
             tc.tile_pool(name="x", bufs=4) as xpool, \
             tc.tile_pool(name="af", bufs=5) as afpool, \
             tc.tile_pool(name="ah", bufs=3) as ahpool, \
             tc.tile_pool(name="sf", bufs=5) as sfpool, \
             tc.tile_pool(name="sh", bufs=3) as shpool, \
             tc.tile_pool(name="vf", bufs=6) as vfpool, \
             tc.tile_pool(name="df", bufs=4) as dfpool, \
             tc.tile_pool(name="p", bufs=12) as ppool, \
             tc.tile_pool(name="o", bufs=4) as opool, \
             tc.tile_pool(name="ps", bufs=1, space="PSUM") as pspool:

            qb_s = gpool.tile([128, 8], F32)
            nc.sync.dma_start(out=qb_s[:], in_=qb_d[:])

            # x tiles FIRST (tiny; the relu chains need them immediately),
            # then H per k-tile in plane-consumption order — a monolithic H
            # transfer ahead of x was serializing ~20us of startup.
            xs_tiles = []
            for ib in range(IB):
                xs = xpool.tile([128, BC], F16, tag="x", name=f"xs_{ib}")
                nc.sync.dma_start(out=xs[:], in_=xt_d[ib * 128:(ib + 1) * 128, :])
                xs_tiles.append(xs)

            h2_s = gpool.tile([128, KT, O], F16)
            for ib in range(IB):
                for q in (5, 6, 4, 3, 2, 1, 0):
                    kt = ib * Q + q
                    nc.sync.dma_start(
                        out=h2_s[:, kt, :],
                        in_=h2_d[kt * 128:(kt + 1) * 128, :])

            psums = [pspool.tile([128, O], F32, name=f"ps{bb}", tag=f"ps{bb}")
                     for bb in range(NBB)]

            issue = {"n": 0}

            def mk_plane(ib, q, lhs):
                kt = ib * Q + q
                rhs = h2_s[:, kt, :]
                first = issue["n"] == 0
                last = issue["n"] == KT - 1
                issue["n"] += 1
                for bb in range(NBB):
                    nc.tensor.matmul(psums[bb][:],
                                     lhs[:, bb * 128:(bb + 1) * 128],
                                     rhs,
                                     start=first, stop=last)
                    if last:
                        o = opool.tile([128, O], F32, tag="o")
                        if bb % 2 == 0:
                            nc.scalar.copy(o[:], psums[bb][:])
                        else:
                            nc.vector.tensor_copy(out=o[:], in_=psums[bb][:])
                        nc.sync.dma_start(
                            out=out_d[bb * 128:(bb + 1) * 128, :], in_=o[:])

            def head(ib):
                """Issue all ops of i-block ib that do not depend on the
                slow GpSimd D1 chain: fp16 planes q=5,6,4, the fp16-rounded
                V3 plane, the f32 cubes, and the GP D1 subs themselves."""
                xs = xs_tiles[ib]

                def relu(q, dtype):
                    a = (afpool if dtype == F32 else ahpool).tile(
                        [128, BC], dtype, tag="a" + ("f" if dtype == F32 else "h"))
                    nc.scalar.activation(a[:], xs[:], ACTF.Relu,
                                         bias=qb_s[:, q:q + 1], scale=1.0)
                    return a

                def sq_act(a, dtype):
                    sq = (sfpool if dtype == F32 else shpool).tile(
                        [128, BC], dtype, tag="s" + ("f" if dtype == F32 else "h"))
                    nc.scalar.activation(sq[:], a[:], ACTF.Square)
                    return sq

                def cube16(q, sq_on_act=False):
                    a = relu(q, F16)
                    if sq_on_act:
                        sq = sq_act(a, F16)
                    else:
                        sq = shpool.tile([128, BC], F16, tag="sh")
                        nc.vector.tensor_tensor(out=sq[:], in0=a[:], in1=a[:],
                                                op=AOT.mult)
                    vq = ppool.tile([128, BC], F16, tag="pl", bufs=12,
                                    name=f"v16_{ib}_{q}")
                    nc.vector.tensor_tensor(out=vq[:], in0=sq[:], in1=a[:],
                                            op=AOT.mult)
                    return vq

                def cube32(q, out_dtype=F32, sq_on_act=False):
                    """f32 relu+square; cube on DVE (f32 or fp16-rounded out)."""
                    a = relu(q, F32)
                    if sq_on_act:
                        sq = sq_act(a, F32)
                    else:
                        sq = sfpool.tile([128, BC], F32, tag="sf")
                        nc.vector.tensor_tensor(out=sq[:], in0=a[:], in1=a[:],
                                                op=AOT.mult)
                    if out_dtype == F32:
                        vq = vfpool.tile([128, BC], F32, tag="vf", bufs=6,
                                         name=f"v32_{ib}_{q}")
                    else:
                        vq = ppool.tile([128, BC], F16, tag="pl", bufs=12,
                                        name=f"v3h_{ib}")
                    nc.vector.tensor_tensor(out=vq[:], in0=sq[:], in1=a[:],
                                            op=AOT.mult)
                    return vq

                def gp_sub(x0, x1, name):
                    d = dfpool.tile([128, BC], F32, tag="df", bufs=7,
                                    name=f"{name}_{ib}")
                    nc.gpsimd.tensor_tensor(out=d[:], in0=x0[:], in1=x1[:],
                                            op=AOT.subtract)
                    return d

                # cheap fp16 planes first so the PE starts early
                mk_plane(ib, 5, cube16(5))
                mk_plane(ib, 6, cube16(6))
                mk_plane(ib, 4, cube16(4))
                # V3: f32 relu (ACT), f32 square (ACT, ready before DVE
                # needs it), fp16-rounded cube IS plane 3 and also feeds
                # d1_2 (quantization 64*2^-12 on |D1_2|<=61 is 3e-4)
                v3h = cube32(3, out_dtype=F16, sq_on_act=True)
                mk_plane(ib, 3, v3h)
                v2 = cube32(2)
                # GP writes the D1_2 plane in fp16 directly (|D1_2| <= 61)
                p2 = ppool.tile([128, BC], F16, tag="pl", bufs=12,
                                name=f"p2_{ib}")
                nc.gpsimd.tensor_tensor(out=p2[:], in0=v2[:], in1=v3h[:],
                                        op=AOT.subtract)
                v1 = cube32(1, sq_on_act=True)
                d1_1 = gp_sub(v1, v2, "d1_1")
                v0 = cube32(0, sq_on_act=True)
                d1_0 = gp_sub(v0, v1, "d1_0")
                return ib, p2, d1_0, d1_1

            def tail(st):
                """Fold planes of an earlier i-block, issued after the next
                head so neither DVE nor the PE stalls on the GP chain."""
                ib, p2, d1_0, d1_1 = st
                mk_plane(ib, 2, p2)
                p1 = ppool.tile([128, BC], F16, tag="pl", bufs=12,
                                name=f"p1_{ib}")
                nc.vector.tensor_tensor(out=p1[:], in0=d1_1[:], in1=p2[:],
                                        op=AOT.subtract)
                mk_plane(ib, 1, p1)
                p0 = ppool.tile([128, BC], F16, tag="pl", bufs=12,
                                name=f"p0_{ib}")
                nc.vector.tensor_tensor(out=p0[:], in0=d1_0[:], in1=d1_1[:],
                                        op=AOT.subtract)
                mk_plane(ib, 0, p0)

            prev = None
            for ib in range(IB):
                st = head(ib)
                if prev is not None:
                    tail(prev)
                prev = st
            tail(prev)

    nc.compile()
    return nc


def _get_program(knobs=0):
    if knobs not in _programs:
        _programs[knobs] = _build_program(knobs)
    return _programs[knobs]


_STENS = {0: (1.0,), 1: (1.0, -1.0), 2: (1.0, -2.0, 1.0)}


def _host_prep(x, weights, coefficients, levels=LEVELS):
    x = np.ascontiguousarray(np.asarray(x, dtype=np.float32))
    weights = np.asarray(weights, dtype=np.float32)
    coefficients = np.asarray(coefficients, dtype=np.float32)

    # raw truncated-power coefficients G_q = sum_g w5[q-g]/6 * C2_g  (g<=6)
    c2 = coefficients.astype(np.float64) * weights.astype(np.float64)[:, :, None]
    c2 = c2.transpose(2, 1, 0)[:Q]                 # [7, I, O]
    w5 = np.array([1.0, -4.0, 6.0, -4.0, 1.0])
    graw = np.zeros((Q, I, O), dtype=np.float64)
    for q in range(Q):
        for g in range(Q):
            r = q - g
            if 0 <= r <= 4:
                graw[q] += (w5[r] / 6.0) * c2[g]
    # planes P = A V  =>  coefficients H = A^{-T} G (exact basis change)
    A = np.zeros((Q, Q))
    for q in range(Q):
        for u, s in enumerate(_STENS[levels[q]]):
            if q + u < Q:
                A[q, q + u] = s
    h = np.einsum('pq,qio->pio', np.linalg.inv(A).T, graw)
    # device row order kt = ib*7 + q
    h2k = np.empty((KT, 128, O), dtype=np.float16)
    for ib in range(IB):
        for q in range(Q):
            h2k[ib * Q + q] = h[q, ib * 128:(ib + 1) * 128, :]
    h2k = np.ascontiguousarray(h2k.reshape(KT * 128, O))

    # tpc = clip(3.5*x, -3.5, 3.5) in t-minus-3.5 coords; relu bias is 3.5-q
    tpc = np.clip(3.5 * x, -3.5, 3.5).astype(np.float16)
    xt = np.ascontiguousarray(tpc.T)               # [I, B] fp16
    qb = np.tile((3.5 - np.arange(8, dtype=np.float32))[None, :], (128, 1))

    in_maps = []
    for c in range(N_CORES):
        in_maps.append({
            "xt": np.ascontiguousarray(xt[:, c * BC:(c + 1) * BC]),
            "h2": h2k,
            "qb": qb,
        })
    return in_maps


def _run(x, weights, coefficients, knobs=0, **spmd_kwargs):
    nc = _get_program(knobs)
    in_maps = _host_prep(x, weights, coefficients)
    res = run_bass_kernel_spmd(nc, in_maps, list(range(N_CORES)), **spmd_kwargs)
    out = np.concatenate([res.results[c]["out"] for c in range(N_CORES)], axis=0)
    return out.astype(np.float32), res


def kernel(x, weights, coefficients):
    out, _ = _run(x, weights, coefficients)
    return out


# revision 16
# speedup vs baseline: 1.1876x; 1.0499x over previous
"""Trainium2 Bass kernel for nn_CustomBSplineLayer.

Math: out[b,o] = sum_{i,g} coeff[o,i,g] * w[o,i] * s_g(clip(x[b,i], -1, 1))
where s_g is a cubic B-spline basis (integer knots in t = 3.5*(x+1) space).

Truncated-power identity: V_q = relu(t-q)^3 (q=0..6) spans all s_g on [0,7],
so out = sum_{q,i} P_q[b,i] * H[(q,i), o] for any plane basis P spanning
{V_q} (H solved exactly on host).  Plane basis (levels 2,2,1,0,0,0,0):
    P = {W2_0, W2_1, D1_2, V_3, V_4, V_5, V_6}     (max magnitudes
    36, 30, 61, 64, 27, 8, 1)
with D1_q = V_q - V_{q+1}, W2_q = D1_q - D1_{q+1}.

Precision design (gate is 2e-2; this lands ~4.7e-3 in exact simulation):
  - planes and H are fp16: fp16 x fp16 matmul products are EXACT in f32
    PSUM (PE full rate, same as f32r, minus f32r's 2^-12 product rounding).
  - the fold chain q<=3 (big cube values, catastrophic cancellation if
    quantized early) stays fp32 end-to-end; only the final small-magnitude
    plane values round to fp16.
  - the q>=4 cubes run all-fp16 (V = a*a*a with a = relu(t-q) small), which
    makes those DVE ops 2-4x faster (dve 2x/4x perf modes need 2-byte
    dtypes).
  - tpc = clip(3.5 x, -3.5, 3.5) fp16 on host; its quantization enters all
    planes through the same delta-t, and every plane has small d/dt.

Engine split (per i-block): ACT does 7 relus + 2 squares + 1 cast copy,
DVE does 2 f32 squares + 4 f32 cubes + 2 f32->f16 fold subs + 1 cast +
6 fp16 square/cube ops, GpSimd does the 3 f32 D1 subs.  PSUM drains
alternate ACT/DVE.  Production order q = 5,6,4,3,2,1,0 puts cheap fp16
planes first so the PE starts ~2.5us in and then never starves — the PE
only reaches its full 2.4 GHz pstate after ~3us of CONTINUOUS execution,
so any gap halves matmul throughput.

Layout (data-parallel over batch, 8 cores x 1024 rows):
  - xt [512 i, 1024 b] fp16 per core (i on partitions), relu bias 3.5-q.
  - planes per (i-block, q): [128, 1024] fp16; matmul lhsT slices are
    [128 K, 128 M=batch]; rhs H tiles [128, 512 o] fp16.
  - PSUM [128 b, 512 o] f32 x 8 banks accumulate all 28 k-tiles; each bank
    drains right after its final matmul.
"""

import numpy as np

import concourse.mybir as mybir
from concourse import bacc
import concourse.tile as tile
from concourse.bass_utils import run_bass_kernel_spmd

F32 = mybir.dt.float32
F16 = mybir.dt.float16
AOT = mybir.AluOpType
ACTF = mybir.ActivationFunctionType

N_CORES = 8
BATCH, I, O, G = 8192, 512, 512, 8
BC = BATCH // N_CORES          # 1024 batch rows per core
Q = 7                          # planes q = 0..6
IB = I // 128                  # 4 i-blocks
KT = Q * IB                    # 28 k-tiles of 128
NBB = BC // 128                # 8 batch blocks of 128

LEVELS = (2, 2, 1, 0, 0, 0, 0)

_programs = {}


def _build_program(knobs=0):
    nc = bacc.Bacc("TRN2", target_bir_lowering=False, debug=False,
                   num_devices=N_CORES)
    xt_d = nc.dram_tensor("xt", [I, BC], F16, kind="ExternalInput").ap()
    h2_d = nc.dram_tensor("h2", [KT * 128, O], F16, kind="ExternalInput").ap()
    qb_d = nc.dram_tensor("qb", [128, 8], F32, kind="ExternalInput").ap()
    out_d = nc.dram_tensor("out", [BC, O], F32, kind="ExternalOutput").ap()

    with tile.TileContext(nc) as tc:
        with tc.tile_pool(name="g", bufs=1) as gpool, \
             tc.tile_pool(name="x", bufs=4) as xpool, \
             tc.tile_pool(name="af", bufs=5) as afpool, \
             tc.tile_pool(name="ah", bufs=3) as ahpool, \
             tc.tile_pool(name="sf", bufs=5) as sfpool, \
             tc.tile_pool(name="sh", bufs=3) as shpool, \
             tc.tile_pool(name="vf", bufs=6) as vfpool, \
             tc.tile_pool(name="df", bufs=4) as dfpool, \
             tc.tile_pool(name="p", bufs=12) as ppool, \
             tc.tile_pool(name="o", bufs=4) as opool, \
             tc.tile_pool(name="ps", bufs=1, space="PSUM") as pspool:

            qb_s = gpool.tile([128, 8], F32)
            nc.sync.dma_start(out=qb_s[:], in_=qb_d[:])

            # x tiles FIRST (tiny; the relu chains need them immediately),
            # then H per k-tile in plane-consumption order — a monolithic H
            # transfer ahead of x was serializing ~20us of startup.
            xs_tiles = []
            for ib in range(IB):
                xs = xpool.tile([128, BC], F16, tag="x", name=f"xs_{ib}")
                nc.sync.dma_start(out=xs[:], in_=xt_d[ib * 128:(ib + 1) * 128, :])
                xs_tiles.append(xs)

            h2_s = gpool.tile([128, KT, O], F16)
            for ib in range(IB):
                for q in (5, 6, 4, 3, 2, 1, 0):
                    kt = ib * Q + q
                    nc.sync.dma_start(
                        out=h2_s[:, kt, :],
                        in_=h2_d[kt * 128:(kt + 1) * 128, :])

            psums = [pspool.tile([128, O], F32, name=f"ps{bb}", tag=f"ps{bb}")
                     for bb in range(NBB)]

            issue = {"n": 0}

            def mk_plane(ib, q, lhs):
                kt = ib * Q + q
                rhs = h2_s[:, kt, :]
                first = issue["n"] == 0
                last = issue["n"] == KT - 1
                issue["n"] += 1
                for bb in range(NBB):
                    nc.tensor.matmul(psums[bb][:],
                                     lhs[:, bb * 128:(bb + 1) * 128],
                                     rhs,
                                     start=first, stop=last)
                    if last:
                        # ACT is idle at the end of the kernel; DVE is not
                        o = opool.tile([128, O], F32, tag="o")
                        nc.scalar.copy(o[:], psums[bb][:])
                        nc.sync.dma_start(
                            out=out_d[bb * 128:(bb + 1) * 128, :], in_=o[:])

            def head(ib):
                """Issue all ops of i-block ib that do not depend on the
                slow GpSimd D1 chain: fp16 planes q=5,6,4, the fp16-rounded
                V3 plane, the f32 cubes, and the GP D1 subs themselves."""
                xs = xs_tiles[ib]

                def relu(q, dtype):
                    a = (afpool if dtype == F32 else ahpool).tile(
                        [128, BC], dtype, tag="a" + ("f" if dtype == F32 else "h"))
                    nc.scalar.activation(a[:], xs[:], ACTF.Relu,
                                         bias=qb_s[:, q:q + 1], scale=1.0)
                    return a

                def sq_act(a, dtype):
                    sq = (sfpool if dtype == F32 else shpool).tile(
                        [128, BC], dtype, tag="s" + ("f" if dtype == F32 else "h"))
                    nc.scalar.activation(sq[:], a[:], ACTF.Square)
                    return sq

                def cube16(q, sq_on_act=False):
                    a = relu(q, F16)
                    if sq_on_act:
                        sq = sq_act(a, F16)
                    else:
                        sq = shpool.tile([128, BC], F16, tag="sh")
                        nc.vector.tensor_tensor(out=sq[:], in0=a[:], in1=a[:],
                                                op=AOT.mult)
                    vq = ppool.tile([128, BC], F16, tag="pl", bufs=12,
                                    name=f"v16_{ib}_{q}")
                    nc.vector.tensor_tensor(out=vq[:], in0=sq[:], in1=a[:],
                                            op=AOT.mult)
                    return vq

                def cube32(q, out_dtype=F32, sq_on_act=False):
                    """f32 relu+square; cube on DVE (f32 or fp16-rounded out)."""
                    a = relu(q, F32)
                    if sq_on_act:
                        sq = sq_act(a, F32)
                    else:
                        sq = sfpool.tile([128, BC], F32, tag="sf")
                        nc.vector.tensor_tensor(out=sq[:], in0=a[:], in1=a[:],
                                                op=AOT.mult)
                    if out_dtype == F32:
                        vq = vfpool.tile([128, BC], F32, tag="vf", bufs=6,
                                         name=f"v32_{ib}_{q}")
                    else:
                        vq = ppool.tile([128, BC], F16, tag="pl", bufs=12,
                                        name=f"v3h_{ib}")
                    nc.vector.tensor_tensor(out=vq[:], in0=sq[:], in1=a[:],
                                            op=AOT.mult)
                    return vq

                def gp_sub(x0, x1, name):
                    d = dfpool.tile([128, BC], F32, tag="df", bufs=7,
                                    name=f"{name}_{ib}")
                    nc.gpsimd.tensor_tensor(out=d[:], in0=x0[:], in1=x1[:],
                                            op=AOT.subtract)
                    return d

                # cheap fp16 planes first so the PE starts early.  In the
                # late i-blocks ACT has drained its relu queue and idles,
                # so its spare capacity takes the fp16 squares there (in
                # early i-blocks that would stall DVE on cross-engine deps).
                late = ib >= 2
                mk_plane(ib, 5, cube16(5, sq_on_act=late))
                mk_plane(ib, 6, cube16(6, sq_on_act=late))
                mk_plane(ib, 4, cube16(4, sq_on_act=late))
                # V3: f32 relu (ACT), f32 square (ACT, ready before DVE
                # needs it), fp16-rounded cube IS plane 3 and also feeds
                # d1_2 (quantization 64*2^-12 on |D1_2|<=61 is 3e-4)
                v3h = cube32(3, out_dtype=F16, sq_on_act=True)
                mk_plane(ib, 3, v3h)
                v2 = cube32(2, sq_on_act=late)
                # GP writes the D1_2 plane in fp16 directly (|D1_2| <= 61)
                p2 = ppool.tile([128, BC], F16, tag="pl", bufs=12,
                                name=f"p2_{ib}")
                nc.gpsimd.tensor_tensor(out=p2[:], in0=v2[:], in1=v3h[:],
                                        op=AOT.subtract)
                v1 = cube32(1, sq_on_act=True)
                d1_1 = gp_sub(v1, v2, "d1_1")
                v0 = cube32(0, sq_on_act=True)
                d1_0 = gp_sub(v0, v1, "d1_0")
                return ib, p2, d1_0, d1_1

            def tail(st):
                """Fold planes of an earlier i-block, issued after the next
                head so neither DVE nor the PE stalls on the GP chain."""
                ib, p2, d1_0, d1_1 = st
                mk_plane(ib, 2, p2)
                p1 = ppool.tile([128, BC], F16, tag="pl", bufs=12,
                                name=f"p1_{ib}")
                nc.vector.tensor_tensor(out=p1[:], in0=d1_1[:], in1=p2[:],
                                        op=AOT.subtract)
                mk_plane(ib, 1, p1)
                p0 = ppool.tile([128, BC], F16, tag="pl", bufs=12,
                                name=f"p0_{ib}")
                nc.vector.tensor_tensor(out=p0[:], in0=d1_0[:], in1=d1_1[:],
                                        op=AOT.subtract)
                mk_plane(ib, 0, p0)

            prev = None
            for ib in range(IB):
                st = head(ib)
                if prev is not None:
                    tail(prev)
                prev = st
            tail(prev)

    nc.compile()
    return nc


def _get_program(knobs=0):
    if knobs not in _programs:
        _programs[knobs] = _build_program(knobs)
    return _programs[knobs]


_STENS = {0: (1.0,), 1: (1.0, -1.0), 2: (1.0, -2.0, 1.0)}


def _host_prep(x, weights, coefficients, levels=LEVELS):
    x = np.ascontiguousarray(np.asarray(x, dtype=np.float32))
    weights = np.asarray(weights, dtype=np.float32)
    coefficients = np.asarray(coefficients, dtype=np.float32)

    # raw truncated-power coefficients G_q = sum_g w5[q-g]/6 * C2_g  (g<=6)
    c2 = coefficients.astype(np.float64) * weights.astype(np.float64)[:, :, None]
    c2 = c2.transpose(2, 1, 0)[:Q]                 # [7, I, O]
    w5 = np.array([1.0, -4.0, 6.0, -4.0, 1.0])
    graw = np.zeros((Q, I, O), dtype=np.float64)
    for q in range(Q):
        for g in range(Q):
            r = q - g
            if 0 <= r <= 4:
                graw[q] += (w5[r] / 6.0) * c2[g]
    # planes P = A V  =>  coefficients H = A^{-T} G (exact basis change)
    A = np.zeros((Q, Q))
    for q in range(Q):
        for u, s in enumerate(_STENS[levels[q]]):
            if q + u < Q:
                A[q, q + u] = s
    h = np.einsum('pq,qio->pio', np.linalg.inv(A).T, graw)
    # device row order kt = ib*7 + q
    h2k = np.empty((KT, 128, O), dtype=np.float16)
    for ib in range(IB):
        for q in range(Q):
            h2k[ib * Q + q] = h[q, ib * 128:(ib + 1) * 128, :]
    h2k = np.ascontiguousarray(h2k.reshape(KT * 128, O))

    # tpc = clip(3.5*x, -3.5, 3.5) in t-minus-3.5 coords; relu bias is 3.5-q
    tpc = np.clip(3.5 * x, -3.5, 3.5).astype(np.float16)
    xt = np.ascontiguousarray(tpc.T)               # [I, B] fp16
    qb = np.tile((3.5 - np.arange(8, dtype=np.float32))[None, :], (128, 1))

    in_maps = []
    for c in range(N_CORES):
        in_maps.append({
            "xt": np.ascontiguousarray(xt[:, c * BC:(c + 1) * BC]),
            "h2": h2k,
            "qb": qb,
        })
    return in_maps


def _run(x, weights, coefficients, knobs=0, **spmd_kwargs):
    nc = _get_program(knobs)
    in_maps = _host_prep(x, weights, coefficients)
    res = run_bass_kernel_spmd(nc, in_maps, list(range(N_CORES)), **spmd_kwargs)
    out = np.concatenate([res.results[c]["out"] for c in range(N_CORES)], axis=0)
    return out.astype(np.float32), res


def kernel(x, weights, coefficients):
    out, _ = _run(x, weights, coefficients)
    return out
